# revision 1
# baseline (speedup 1.0000x reference)
"""GCN (3-layer GCNConv + global_add_pool + linear head) on 8 Trainium2 cores.

Strategy:
 - Nodes sharded across 8 cores on graph-id-aligned boundaries (pooling local).
 - Edges partitioned by dst owner. Per core, edges ordered chunk-major
   (src_row % 4 -> int16 gather index fits), then by 128-node dst window,
   padded per (chunk, window) group to multiples of 128 and uniform tile
   counts across cores (single SPMD program).
 - Per layer: dense z = h @ W on PE (transpose-on-the-fly), zn = z * dinv,
   AllGather zn -> full table in DRAM, dma_gather 256B rows per edge,
   segment-sum via one-hot matmul into PSUM per window, accumulated in SBUF
   across the 4 chunk passes, then tanh(dinv*acc + b) in place.
 - Pooling: one-hot(graph id) matmul into a [64, 512] PSUM tile; final
   linear head + tanh on device.
"""

import hashlib
import sys

for _p in ("/opt/trn_rl_repo",):
    if _p not in sys.path:
        sys.path.insert(0, _p)

import numpy as np

P = 128
WIN = 128          # dst-window width (nodes)
NCHUNK = 4         # src chunks (int16 index limit: 8S/4 <= 32767)
GRANULE = 4096     # idxs per dma_gather instruction (ring: 3 in flight)
N_CORES = 8
N_GRAPHS = 2048    # problem constant
GMAX = 512         # per-core graph-count upper bound (psum free dim)


# ----------------------------------------------------------------------------
# Host-side sharding / edge bucketing (index manipulation only, no float math)
# ----------------------------------------------------------------------------

def _preprocess(edge_index, batch, n_nodes, n_graphs):
    C = N_CORES
    src = np.asarray(edge_index[0], dtype=np.int64)
    dst = np.asarray(edge_index[1], dtype=np.int64)
    batch = np.asarray(batch, dtype=np.int64)
    N = n_nodes

    # graph-aligned node shard boundaries
    gstart = np.searchsorted(batch, np.arange(n_graphs + 1))  # [G+1], gstart[G] = N
    node_bnds = [0]
    g_bnds = [0]
    for c in range(1, C):
        tgt = (c * N) // C
        g = int(np.searchsorted(gstart, tgt))
        # candidates g-1, g: pick nearest boundary node
        if g > 0 and abs(int(gstart[g - 1]) - tgt) <= abs(int(gstart[min(g, n_graphs)]) - tgt):
            g = g - 1
        g = min(max(g, g_bnds[-1]), n_graphs)
        g_bnds.append(g)
        node_bnds.append(int(gstart[g]))
    node_bnds.append(N)
    g_bnds.append(n_graphs)
    node_bnds = np.array(node_bnds, dtype=np.int64)          # [C+1]
    g_bnds = np.array(g_bnds, dtype=np.int64)                # [C+1]
    node_cnt = node_bnds[1:] - node_bnds[:-1]
    g_cnt = g_bnds[1:] - g_bnds[:-1]
    assert g_cnt.max() < GMAX - 1, g_cnt

    S = int(-(-node_cnt.max() // P) * P)                     # padded shard size
    NW = S // WIN                                            # windows per core
    assert 2 * S <= 32767, S                                 # int16 gather idx bound

    owner = np.searchsorted(node_bnds[1:], np.arange(N), side="right")
    local = np.arange(N) - node_bnds[owner]
    row = owner * S + local                                  # table row per node

    deg = np.bincount(dst, minlength=N).astype(np.float32) + 1.0

    # edge stream (+ self loops)
    e_src = np.concatenate([src, np.arange(N)])
    e_dst = np.concatenate([dst, np.arange(N)])
    e_owner = owner[e_dst]
    e_dl = local[e_dst]
    e_row = row[e_src]
    e_chunk = (e_row & (NCHUNK - 1)).astype(np.int64)
    e_idx = (e_row >> 2).astype(np.int16)
    e_win = e_dl >> 7

    key = (e_chunk * NW + e_win) * C + e_owner               # chunk-major, then window
    order = np.argsort(key, kind="stable")
    cnt = np.bincount(key, minlength=NCHUNK * NW * C).reshape(NCHUNK, NW, C)

    tiles_kw = -(-cnt.max(axis=2) // P)                      # [NCHUNK, NW] uniform tiles
    # chunk-0 pass initializes the SBUF accumulator (copy): force >=1 tile/window
    tiles_kw[0] = np.maximum(tiles_kw[0], 1)
    pad_kw = tiles_kw * P                                    # padded group sizes
    E_PAD = int(pad_kw.sum())
    # group start offsets in the uniform stream (same for all cores)
    goff = np.zeros((NCHUNK, NW), dtype=np.int64)
    goff.flat[1:] = np.cumsum(pad_kw.flat)[:-1]

    idx16 = np.zeros((C, E_PAD), dtype=np.int16)             # pad -> idx 0 (valid row)
    dstl = np.full((C, E_PAD), -1.0, dtype=np.float32)       # pad -> -1 (one-hot miss)

    # place real edges
    so = order
    r_owner = e_owner[so]
    r_chunk = e_chunk[so]
    r_win = e_win[so]
    # position within (chunk, win, owner) group = running index
    rkey = (r_chunk * NW + r_win) * C + r_owner
    # stable sort => positions are 0..cnt-1 in order of appearance
    pos = np.zeros(len(so), dtype=np.int64)
    _, first_idx, inv = np.unique(rkey, return_index=True, return_inverse=True)
    pos = np.arange(len(so)) - first_idx[inv]
    slot = goff[r_chunk, r_win] + pos
    idx16[r_owner, slot] = e_idx[so]
    dstl[r_owner, slot] = (e_dl[so] - r_win * WIN).astype(np.float32)

    # per-tile metadata (uniform): window id, group-first, group-last
    TILES = E_PAD // P
    tile_win = np.zeros(TILES, dtype=np.int64)
    tile_first = np.zeros(TILES, dtype=bool)
    tile_last = np.zeros(TILES, dtype=bool)
    for k in range(NCHUNK):
        for w in range(NW):
            t0 = goff[k, w] // P
            nt = int(tiles_kw[k, w])
            if nt == 0:
                continue
            tile_win[t0:t0 + nt] = w
            tile_first[t0] = True
            tile_last[t0 + nt - 1] = True
    # chunk segment boundaries (in idx positions)
    chunk_off = [int(goff[k, 0]) for k in range(NCHUNK)] + [E_PAD]

    # gather-layout idx: [16, E_PAD/16] with [p, s] = stream[s*16+p]
    idx_wrapped = np.ascontiguousarray(
        idx16.reshape(C, E_PAD // 16, 16).transpose(0, 2, 1))
    # dstl layout [128, E_PAD/128] with [p, t] = stream[t*128+p]
    dstl_wrapped = np.ascontiguousarray(
        dstl.reshape(C, TILES, P).transpose(0, 2, 1))

    # per-core padded node arrays
    deg_pad = np.ones((C, S), dtype=np.float32)
    batchl = np.full((C, S), float(GMAX - 1), dtype=np.float32)
    for c in range(C):
        n0, n1 = int(node_bnds[c]), int(node_bnds[c + 1])
        deg_pad[c, : n1 - n0] = deg[n0:n1]
        batchl[c, : n1 - n0] = (batch[n0:n1] - g_bnds[c]).astype(np.float32)
    batchl_wrapped = np.ascontiguousarray(
        batchl.reshape(C, NW, P).transpose(0, 2, 1))         # [C, 128, NW]

    return dict(
        S=S, NW=NW, E_PAD=E_PAD, TILES=TILES,
        node_bnds=node_bnds, g_bnds=g_bnds, node_cnt=node_cnt, g_cnt=g_cnt,
        idx_wrapped=idx_wrapped, dstl_wrapped=dstl_wrapped,
        batchl_wrapped=batchl_wrapped, deg_pad=deg_pad,
        tile_win=tile_win, tile_first=tile_first, tile_last=tile_last,
        chunk_off=chunk_off,
    )


# ----------------------------------------------------------------------------
# Bass program builder
# ----------------------------------------------------------------------------

def _build_program(meta, d_in, h_dim, n_cls):
    import concourse.bacc as bacc
    import concourse.mybir as mybir
    import concourse.tile as tile
    from concourse import library_config

    S, NW, E_PAD = meta["S"], meta["NW"], meta["E_PAD"]
    tile_win = meta["tile_win"]
    tile_first = meta["tile_first"]
    tile_last = meta["tile_last"]
    chunk_off = meta["chunk_off"]
    f32 = mybir.dt.float32
    AOT = mybir.ActivationFunctionType
    ALU = mybir.AluOpType

    nc = bacc.Bacc("TRN2", target_bir_lowering=False, debug=False,
                   num_devices=N_CORES)

    # --- I/O ---
    x_d = nc.dram_tensor("x_loc", [S, d_in], f32, kind="ExternalInput").ap()
    deg_d = nc.dram_tensor("deg_loc", [S], f32, kind="ExternalInput").ap()
    idx_d = nc.dram_tensor("idx16", [P, E_PAD // 16], mybir.dt.int16,
                           kind="ExternalInput").ap()
    dstl_d = nc.dram_tensor("dstl", [P, E_PAD // P], f32,
                            kind="ExternalInput").ap()
    batchl_d = nc.dram_tensor("batchl", [P, NW], f32, kind="ExternalInput").ap()
    W_d = [nc.dram_tensor("W1", [d_in, h_dim], f32, kind="ExternalInput").ap(),
           nc.dram_tensor("W2", [h_dim, h_dim], f32, kind="ExternalInput").ap(),
           nc.dram_tensor("W3", [h_dim, h_dim], f32, kind="ExternalInput").ap()]
    Wf_d = nc.dram_tensor("Wf", [h_dim, n_cls], f32, kind="ExternalInput").ap()
    b_d = [nc.dram_tensor(f"b{i+1}b", [P, h_dim], f32, kind="ExternalInput").ap()
           for i in range(3)]
    bf_d = nc.dram_tensor("bfb", [P, n_cls], f32, kind="ExternalInput").ap()
    out_d = nc.dram_tensor("out", [GMAX, n_cls], f32, kind="ExternalOutput").ap()
    ident_d = nc.dram_tensor("ident", [P, P], f32, kind="ExternalInput").ap()
    iota_w_d = nc.dram_tensor("iota_w", [P, WIN], f32, kind="ExternalInput").ap()
    iota_g_d = nc.dram_tensor("iota_g", [P, GMAX], f32, kind="ExternalInput").ap()

    zn_d = nc.dram_tensor("zn_loc", [S, h_dim], f32).ap()
    table_d = nc.dram_tensor("table", [N_CORES * S, h_dim], f32,
                             addr_space="Shared").ap()
    chunk_views = table_d.rearrange("(n four) d -> four n d", four=NCHUNK)
    rg = [list(range(N_CORES))]

    with tile.TileContext(nc) as tc:
        with (
            tc.tile_pool(name="persist", bufs=1) as pp,
            tc.tile_pool(name="msg", bufs=4) as msgp,
            tc.tile_pool(name="work", bufs=4) as wp,
            tc.tile_pool(name="dense", bufs=3) as dp,
            tc.tile_pool(name="psum", bufs=2, space="PSUM") as psp,
            tc.tile_pool(name="psum1", bufs=2, space="PSUM") as ps1,
            tc.tile_pool(name="pool_ps", bufs=1, space="PSUM") as poolps,
        ):
            # --- persistent tiles ---
            nc.gpsimd.load_library(library_config.mlp)
            ident = pp.tile([P, P], f32, tag="ident")
            nc.sync.dma_start(ident[:], ident_d[:])
            iota_w = pp.tile([P, WIN], f32, tag="iota_w")
            nc.sync.dma_start(iota_w[:], iota_w_d[:])
            iota_g = pp.tile([P, GMAX], f32, tag="iota_g")
            nc.sync.dma_start(iota_g[:], iota_g_d[:])

            W_sb = []
            for i in range(3):
                k = d_in if i == 0 else h_dim
                t = pp.tile([k, h_dim], f32, tag=f"W{i}")
                nc.sync.dma_start(t[:], W_d[i][:])
                W_sb.append(t)
            Wf_sb = pp.tile([h_dim, n_cls], f32, tag="Wf")
            nc.sync.dma_start(Wf_sb[:], Wf_d[:])
            b_sb = []
            for i in range(3):
                t = pp.tile([P, h_dim], f32, tag=f"b{i}")
                nc.sync.dma_start(t[:], b_d[i][:])
                b_sb.append(t)
            bf_sb = pp.tile([P, n_cls], f32, tag="bf")
            nc.sync.dma_start(bf_sb[:], bf_d[:])

            idx_sb = pp.tile([P, E_PAD // 16], mybir.dt.int16, tag="idx")
            nc.sync.dma_start(idx_sb[:], idx_d[:])
            dstl_sb = pp.tile([P, E_PAD // P], f32, tag="dstl")
            nc.sync.dma_start(dstl_sb[:], dstl_d[:])
            batchl_sb = pp.tile([P, NW], f32, tag="batchl")
            nc.sync.dma_start(batchl_sb[:], batchl_d[:])

            dinv = pp.tile([P, NW], f32, tag="dinv")
            deg_col = pp.tile([P, NW], f32, tag="degc")
            nc.sync.dma_start(deg_col[:], deg_d.rearrange("(t p) -> p t", p=P))
            # dinv = 1/sqrt(deg): sqrt on ACT, then DVE reciprocal
            nc.scalar.activation(deg_col[:], deg_col[:], AOT.Sqrt)
            nc.vector.reciprocal(dinv[:], deg_col[:])

            bufA = pp.tile([P, NW * h_dim], f32, tag="bufA")

            # === 3 GCN layers ===
            for layer in range(3):
                # ---- dense: zn = (h_in @ W) * dinv, tile by tile ----
                for t in range(NW):
                    if layer == 0:
                        xt = dp.tile([P, d_in], f32, tag="xt")
                        nc.sync.dma_start(xt[:], x_d[t * P:(t + 1) * P, :])
                        tp = ps1.tile([d_in, P], f32, tag="tps")
                        nc.tensor.transpose(tp[:], xt[:], ident[:])
                        sbT = dp.tile([d_in, P], f32, tag="sbT")
                        nc.vector.tensor_copy(sbT[:], tp[:])
                        kdim = d_in
                    else:
                        tp = ps1.tile([h_dim, P], f32, tag="tps")
                        nc.tensor.transpose(
                            tp[:], bufA[:, t * h_dim:(t + 1) * h_dim], ident[:])
                        sbT = dp.tile([h_dim, P], f32, tag="sbT")
                        nc.vector.tensor_copy(sbT[:], tp[:])
                        kdim = h_dim
                    zps = ps1.tile([P, h_dim], f32, tag="zps")
                    nc.tensor.matmul(zps[:], lhsT=sbT[:], rhs=W_sb[layer][:],
                                     start=True, stop=True)
                    nc.vector.tensor_scalar(
                        out=bufA[:, t * h_dim:(t + 1) * h_dim], in0=zps[:],
                        scalar1=dinv[:, t:t + 1], scalar2=None, op0=ALU.mult)

                # ---- publish zn + AllGather ----
                nc.sync.dma_start(
                    zn_d.rearrange("(t p) d -> p t d", p=P),
                    bufA[:].rearrange("p (t d) -> p t d", d=h_dim))
                nc.gpsimd.collective_compute(
                    "AllGather", ALU.bypass, replica_groups=rg,
                    ins=[zn_d[:]], outs=[table_d[:]])

                # ---- sparse aggregation: chunk-major gather + one-hot matmul ----
                wpsum = None
                for k in range(NCHUNK):
                    seg0, seg1 = chunk_off[k], chunk_off[k + 1]
                    for a in range(seg0, seg1, GRANULE):
                        gsz = min(GRANULE, seg1 - a)
                        gT = gsz // P
                        msg = msgp.tile([P, gT * h_dim], f32, tag="msg")
                        nc.gpsimd.dma_gather(
                            msg[:].rearrange("p (t d) -> p t d", d=h_dim),
                            chunk_views[k],
                            idx_sb[:, a // 16:(a + gsz) // 16],
                            gsz, gsz, h_dim, elem_step=NCHUNK * h_dim,
                            single_packet=False)
                        for i in range(gT):
                            t = a // P + i
                            w = int(tile_win[t])
                            oh = wp.tile([P, WIN], f32, tag="oh")
                            nc.vector.tensor_scalar(
                                out=oh[:], in0=iota_w[:],
                                scalar1=dstl_sb[:, t:t + 1], scalar2=None,
                                op0=ALU.is_equal)
                            if tile_first[t]:
                                wpsum = psp.tile([WIN, h_dim], f32, tag="wps")
                            nc.tensor.matmul(
                                wpsum[:], lhsT=oh[:],
                                rhs=msg[:, i * h_dim:(i + 1) * h_dim],
                                start=bool(tile_first[t]),
                                stop=bool(tile_last[t]))
                            if tile_last[t]:
                                dst = bufA[:, w * h_dim:(w + 1) * h_dim]
                                if k == 0:
                                    nc.vector.tensor_copy(dst, wpsum[:])
                                else:
                                    nc.vector.tensor_tensor(
                                        out=dst, in0=dst, in1=wpsum[:],
                                        op=ALU.add)

                # ---- flush: h = tanh(dinv * acc + b), in place ----
                for w in range(NW):
                    sl = bufA[:, w * h_dim:(w + 1) * h_dim]
                    tmp = wp.tile([P, h_dim], f32, tag="ftmp")
                    nc.vector.tensor_scalar(
                        out=tmp[:], in0=sl, scalar1=dinv[:, w:w + 1],
                        scalar2=None, op0=ALU.mult)
                    nc.vector.tensor_tensor(out=tmp[:], in0=tmp[:],
                                            in1=b_sb[layer][:], op=ALU.add)
                    nc.scalar.activation(sl, tmp[:], AOT.Tanh)

            # === pooling: pooledT[64, GMAX] = sum_h3 by graph ===
            poolT = poolps.tile([h_dim, GMAX], f32, tag="poolT")
            for t in range(NW):
                ohg = wp.tile([P, GMAX], f32, tag="ohg")
                nc.vector.tensor_scalar(
                    out=ohg[:], in0=iota_g[:], scalar1=batchl_sb[:, t:t + 1],
                    scalar2=None, op0=ALU.is_equal)
                nc.tensor.matmul(poolT[:],
                                 lhsT=bufA[:, t * h_dim:(t + 1) * h_dim],
                                 rhs=ohg[:], start=(t == 0), stop=(t == NW - 1))
            poolS = pp.tile([h_dim, GMAX], f32, tag="poolS")
            nc.vector.tensor_copy(poolS[:], poolT[:])

            # === head: out = tanh(pooled @ Wf + bf) ===
            for gt in range(GMAX // P):
                fps = psp.tile([P, n_cls], f32, tag="wps")
                nc.tensor.matmul(fps[:], lhsT=poolS[:, gt * P:(gt + 1) * P],
                                 rhs=Wf_sb[:], start=True, stop=True)
                ot = wp.tile([P, n_cls], f32, tag="ot")
                nc.vector.tensor_tensor(out=ot[:], in0=fps[:], in1=bf_sb[:],
                                        op=ALU.add)
                nc.scalar.activation(ot[:], ot[:], AOT.Tanh)
                nc.sync.dma_start(out_d[gt * P:(gt + 1) * P, :], ot[:])

    nc.compile()
    return nc


# ----------------------------------------------------------------------------
# Runner (persistent compiled program + per-core inputs)
# ----------------------------------------------------------------------------

class Runner:
    def __init__(self, meta, nc, d_in, h_dim, n_cls):
        self.meta = meta
        self.nc = nc
        self.d_in, self.h_dim, self.n_cls = d_in, h_dim, n_cls

    def in_maps(self, x, W1, b1, W2, b2, W3, b3, Wf, bf):
        m = self.meta
        S = m["S"]
        C = N_CORES
        x = np.asarray(x, np.float32)
        maps = []
        reps = dict(
            W1=np.asarray(W1, np.float32), W2=np.asarray(W2, np.float32),
            W3=np.asarray(W3, np.float32), Wf=np.asarray(Wf, np.float32),
            b1b=np.broadcast_to(np.asarray(b1, np.float32), (P, self.h_dim)).copy(),
            b2b=np.broadcast_to(np.asarray(b2, np.float32), (P, self.h_dim)).copy(),
            b3b=np.broadcast_to(np.asarray(b3, np.float32), (P, self.h_dim)).copy(),
            bfb=np.broadcast_to(np.asarray(bf, np.float32), (P, self.n_cls)).copy(),
            ident=np.eye(P, dtype=np.float32),
            iota_w=np.broadcast_to(np.arange(WIN, dtype=np.float32), (P, WIN)).copy(),
            iota_g=np.broadcast_to(np.arange(GMAX, dtype=np.float32), (P, GMAX)).copy(),
        )
        for c in range(C):
            n0, n1 = int(m["node_bnds"][c]), int(m["node_bnds"][c + 1])
            xl = np.zeros((S, self.d_in), np.float32)
            xl[: n1 - n0] = x[n0:n1]
            maps.append(dict(
                x_loc=xl,
                deg_loc=m["deg_pad"][c],
                idx16=np.tile(m["idx_wrapped"][c], (8, 1)),
                dstl=m["dstl_wrapped"][c],
                batchl=m["batchl_wrapped"][c],
                **reps,
            ))
        return maps

    def run(self, maps):
        from concourse.bass_utils import run_bass_kernel_spmd
        res = run_bass_kernel_spmd(self.nc, maps, list(range(N_CORES)))
        return self.assemble(res.results)

    def make_timed(self, maps):
        """Build a callable with inputs resident on device; each call runs the
        NEFF once and returns per-core outputs. For timing (transfer excluded)."""
        import jax
        import concourse.mybir as mybir
        from concourse import bass2jax
        from jax.experimental.shard_map import shard_map
        from jax.sharding import Mesh, NamedSharding, PartitionSpec

        nc = self.nc
        bass2jax.install_neuronx_cc_hook()
        partition_name = (nc.partition_id_tensor.name
                          if nc.partition_id_tensor else None)
        in_names, out_names, out_avals, zero_outs = [], [], [], []
        for alloc in nc.m.functions[0].allocations:
            if not isinstance(alloc, mybir.MemoryLocationSet):
                continue
            name = alloc.memorylocations[0].name
            if alloc.kind == "ExternalInput":
                if name != partition_name:
                    in_names.append(name)
            elif alloc.kind == "ExternalOutput":
                shape = tuple(alloc.tensor_shape)
                dtype = mybir.dt.np(alloc.dtype)
                out_names.append(name)
                out_avals.append(jax.core.ShapedArray(shape, dtype))
                zero_outs.append(np.zeros(shape, dtype))
        n_params = len(in_names)
        all_in = list(in_names) + list(out_names)
        if partition_name is not None:
            all_in.append(partition_name)
        donate = tuple(range(n_params, n_params + len(out_names)))

        def _body(*args):
            operands = list(args)
            if partition_name is not None:
                operands.append(bass2jax.partition_id_tensor())
            return tuple(bass2jax._bass_exec_p.bind(
                *operands, out_avals=tuple(out_avals), in_names=tuple(all_in),
                out_names=tuple(out_names), lowering_input_output_aliases=(),
                sim_require_finite=True, sim_require_nnan=True, nc=nc))

        devices = jax.devices()[:N_CORES]
        mesh = Mesh(np.asarray(devices), ("core",))
        spec = NamedSharding(mesh, PartitionSpec("core"))
        fn = jax.jit(shard_map(_body, mesh=mesh,
                               in_specs=(PartitionSpec("core"),) * (n_params + len(out_names)),
                               out_specs=(PartitionSpec("core"),) * len(out_names)),
                     donate_argnums=donate, keep_unused=True)
        dev_in = [jax.device_put(
            np.concatenate([np.asarray(maps[c][nm]) for c in range(N_CORES)], axis=0),
            spec) for nm in in_names]
        zshapes = [(N_CORES * z.shape[0], *z.shape[1:]) for z in zero_outs]
        zdtypes = [z.dtype for z in zero_outs]

        def call():
            zs = [jax.device_put(np.zeros(s, d), spec)
                  for s, d in zip(zshapes, zdtypes)]
            outs = fn(*dev_in, *zs)
            return [o.block_until_ready() for o in outs]

        return call, out_names, out_avals

    def assemble(self, results):
        m = self.meta
        outs = []
        for c in range(N_CORES):
            outs.append(results[c]["out"][: int(m["g_cnt"][c])])
        return np.concatenate(outs, axis=0)


_CACHE = {}


def _get_runner(edge_index, batch, n_nodes, n_graphs, d_in, h_dim, n_cls):
    key = (hashlib.sha1(np.ascontiguousarray(edge_index).tobytes()).hexdigest(),
           hashlib.sha1(np.ascontiguousarray(batch).tobytes()).hexdigest(),
           n_nodes, n_graphs, d_in, h_dim, n_cls)
    r = _CACHE.get(key)
    if r is None:
        meta = _preprocess(edge_index, batch, n_nodes, n_graphs)
        nc = _build_program(meta, d_in, h_dim, n_cls)
        r = Runner(meta, nc, d_in, h_dim, n_cls)
        _CACHE[key] = r
    return r


def kernel(x, edge_index, batch, W1, b1, W2, b2, W3, b3, Wf, bf):
    x = np.asarray(x)
    r = _get_runner(np.asarray(edge_index), np.asarray(batch), x.shape[0],
                    N_GRAPHS, x.shape[1], np.asarray(W1).shape[1],
                    np.asarray(Wf).shape[1])
    maps = r.in_maps(x, W1, b1, W2, b2, W3, b3, Wf, bf)
    return r.run(maps)



# revision 2
# speedup vs baseline: 2.1703x; 2.1703x over previous
"""GCN (3x GCNConv + global_add_pool + linear head) on 8 Trainium2 cores.

Layout/strategy (v2):
 - Nodes sharded across 8 cores on graph-aligned boundaries (pooling stays
   local); shard padded to S (multiple of 512). Edges partitioned by dst
   owner, bucketed by (dst window, src quarter), padded per group to x128
   uniformly across cores (single SPMD program).
 - Per layer: dense z = h @ W on PE (x shipped pre-transposed, so layer 1
   needs no transposes), zn = z*dinv written per quarter, fp32 AllGather
   per quarter directly into contiguous [8*S/4, 64] gather tables (int16
   dma_gather indices fit a quarter), dma_gather 256B rows per edge spread
   across 2 SWDGE queues, scatter via fp8 one-hot matmuls (precomputed on
   host, streamed from HBM) with bf16 messages, accumulating in PSUM
   across all 4 src-quarter chunks of a window, fused flush
   (scalar_tensor_tensor + tanh on ACT).
 - Pooling: precomputed fp8 graph one-hots, bf16 h3, single PSUM tile.
"""

import hashlib
import sys

for _p in ("/opt/trn_rl_repo",):
    if _p not in sys.path:
        sys.path.insert(0, _p)

import numpy as np

P = 128
H = 64
NQ = 4              # src-quarter chunks
NWQ = 16            # window groups (gather granules per chunk)
N_CORES = 8
N_GRAPHS = 2048
GMAX = 512
D_IN = 128
N_CLS = 10


# ----------------------------------------------------------------------------
# Host-side sharding / edge bucketing (index manipulation only)
# ----------------------------------------------------------------------------

def _preprocess(edge_index, batch, n_nodes, n_graphs):
    C = N_CORES
    src = np.asarray(edge_index[0], dtype=np.int64)
    dst = np.asarray(edge_index[1], dtype=np.int64)
    batch = np.asarray(batch, dtype=np.int64)
    N = n_nodes

    # graph-aligned node shard boundaries
    gstart = np.searchsorted(batch, np.arange(n_graphs + 1))
    node_bnds = [0]
    g_bnds = [0]
    for c in range(1, C):
        tgt = (c * N) // C
        g = int(np.searchsorted(gstart, tgt))
        if g > 0 and abs(int(gstart[g - 1]) - tgt) <= abs(
                int(gstart[min(g, n_graphs)]) - tgt):
            g = g - 1
        g = min(max(g, g_bnds[-1]), n_graphs)
        g_bnds.append(g)
        node_bnds.append(int(gstart[g]))
    node_bnds.append(N)
    g_bnds.append(n_graphs)
    node_bnds = np.array(node_bnds, dtype=np.int64)
    g_bnds = np.array(g_bnds, dtype=np.int64)
    node_cnt = node_bnds[1:] - node_bnds[:-1]
    g_cnt = g_bnds[1:] - g_bnds[:-1]
    assert g_cnt.max() < GMAX, g_cnt

    S = int(-(-node_cnt.max() // (4 * P)) * (4 * P))
    assert 2 * S <= 32767, S
    QROWS = S // NQ
    NW = S // P
    WPQ = NW // NQ

    owner = np.searchsorted(node_bnds[1:], np.arange(N), side="right")
    local = np.arange(N) - node_bnds[owner]

    deg = np.bincount(dst, minlength=N).astype(np.float64) + 1.0
    dinv_g = (1.0 / np.sqrt(deg)).astype(np.float32)

    # edge stream (+ self loops)
    es = np.concatenate([src, np.arange(N)])
    ed = np.concatenate([dst, np.arange(N)])
    e_c = owner[ed]
    e_dl = local[ed]
    e_w = e_dl >> 7
    e_p = e_dl & 127
    e_sl = local[es]
    e_k = e_sl // QROWS
    e_ix = (owner[es] * QROWS + (e_sl % QROWS)).astype(np.int16)

    cnt = np.zeros((C, NQ, NW), dtype=np.int64)
    np.add.at(cnt, (e_c, e_k, e_w), 1)
    pad_kw = (-(-cnt.max(axis=0) // P) * P)          # [NQ, NW]
    tiles_kw = pad_kw // P

    # window groups: consecutive windows, balanced by total tile count
    tiles_w = tiles_kw.sum(axis=0)                   # [NW]
    csum = np.concatenate([[0], np.cumsum(tiles_w)])  # [NW+1]
    tot = int(csum[-1])
    cuts = [0]
    for i in range(1, NWQ):
        tgt = (tot * i) // NWQ
        w = int(np.searchsorted(csum, tgt))
        w = min(max(w, cuts[-1] + 1), NW - (NWQ - i))
        cuts.append(w)
    cuts.append(NW)
    wq_win = [list(range(cuts[i], cuts[i + 1])) for i in range(NWQ)]
    assert all(len(g) for g in wq_win), wq_win

    # gather stream offsets: order (wq, k, w in wq)
    goff = np.zeros((NQ, NW), dtype=np.int64)
    gq_start = np.zeros((NQ, NWQ), dtype=np.int64)
    gq_size = np.zeros((NQ, NWQ), dtype=np.int64)
    cur = 0
    for wq in range(NWQ):
        for k in range(NQ):
            gq_start[k, wq] = cur
            for w in wq_win[wq]:
                goff[k, w] = cur
                cur += int(pad_kw[k, w])
            gq_size[k, wq] = cur - gq_start[k, wq]
    E_PAD = cur
    TILES = E_PAD // P

    # scatter tile offsets: order (wq, w, k)
    sc_tile = np.zeros((NQ, NW), dtype=np.int64)
    oh_t0 = np.zeros(NWQ, dtype=np.int64)
    oh_t1 = np.zeros(NWQ, dtype=np.int64)
    tcur = 0
    for wq in range(NWQ):
        oh_t0[wq] = tcur
        for w in wq_win[wq]:
            for k in range(NQ):
                sc_tile[k, w] = tcur
                tcur += int(tiles_kw[k, w])
        oh_t1[wq] = tcur
    assert tcur == TILES

    # place edges: position within (core, chunk, window) group
    key = (e_k * NW + e_w) * C + e_c
    order = np.argsort(key, kind="stable")
    so_key = key[order]
    _, first_idx, inv = np.unique(so_key, return_index=True, return_inverse=True)
    pos = np.arange(len(order)) - first_idx[inv]
    r_c = e_c[order]
    r_k = e_k[order]
    r_w = e_w[order]
    gslot = goff[r_k, r_w] + pos
    sctile = sc_tile[r_k, r_w] + pos // P
    scrow = pos % P

    idx16 = np.zeros((C, E_PAD), dtype=np.int16)
    idx16[r_c, gslot] = e_ix[order]
    idx_wrapped = np.ascontiguousarray(
        idx16.reshape(C, E_PAD // 16, 16).transpose(0, 2, 1))
    idx_tiled = np.ascontiguousarray(np.tile(idx_wrapped, (1, 8, 1)))  # [C,128,E/16]

    import ml_dtypes
    fp8 = ml_dtypes.float8_e4m3
    one8 = np.ones((), dtype=fp8)
    oh8 = np.zeros((C, TILES, P, P), dtype=fp8)
    oh8[r_c, sctile, scrow, e_p[order]] = one8
    oh8 = np.ascontiguousarray(
        oh8.transpose(0, 2, 1, 3).reshape(C, P, TILES * P))

    # pooling one-hots [C, P, NW*GMAX]
    ohg8 = np.zeros((C, P, NW * GMAX), dtype=fp8)
    for c in range(C):
        n0, n1 = int(node_bnds[c]), int(node_bnds[c + 1])
        nn = n1 - n0
        nodes = np.arange(nn)
        g_loc = (batch[n0:n1] - g_bnds[c]).astype(np.int64)
        w_arr = nodes >> 7
        p_arr = nodes & 127
        ohg8[c, p_arr, w_arr * GMAX + g_loc] = one8

    # dinv per-core [P, NW]
    dinv_col = np.ones((C, P, NW), dtype=np.float32)
    for c in range(C):
        n0, n1 = int(node_bnds[c]), int(node_bnds[c + 1])
        nn = n1 - n0
        v = np.ones(S, dtype=np.float32)
        v[:nn] = dinv_g[n0:n1]
        dinv_col[c] = v.reshape(NW, P).T

    return dict(
        S=S, QROWS=QROWS, NW=NW, WPQ=WPQ, E_PAD=E_PAD, TILES=TILES,
        node_bnds=node_bnds, g_bnds=g_bnds, node_cnt=node_cnt, g_cnt=g_cnt,
        wq_win=wq_win, goff=goff, gq_start=gq_start, gq_size=gq_size,
        tiles_kw=tiles_kw, sc_tile=sc_tile, oh_t0=oh_t0, oh_t1=oh_t1,
        idx_tiled=idx_tiled, oh8=oh8, ohg8=ohg8, dinv_col=dinv_col,
    )


# ----------------------------------------------------------------------------
# Bass program builder
# ----------------------------------------------------------------------------

def _build_program(meta, reps=1, do_ag=True, do_expand=True, do_gather=True,
                   do_scatter=True, n_queues=2, single_packet=False,
                   fp32_ag=True):
    import concourse.bacc as bacc
    import concourse.mybir as mybir
    import concourse.tile as tile
    from concourse import library_config

    S, QROWS, NW, WPQ = meta["S"], meta["QROWS"], meta["NW"], meta["WPQ"]
    E_PAD, TILES = meta["E_PAD"], meta["TILES"]
    wq_win = meta["wq_win"]
    goff, gq_start, gq_size = meta["goff"], meta["gq_start"], meta["gq_size"]
    tiles_kw, sc_tile = meta["tiles_kw"], meta["sc_tile"]
    oh_t0, oh_t1 = meta["oh_t0"], meta["oh_t1"]

    f32 = mybir.dt.float32
    bf16 = mybir.dt.bfloat16
    fp8 = mybir.dt.float8e4
    i16 = mybir.dt.int16
    AOT = mybir.ActivationFunctionType
    ALU = mybir.AluOpType

    GQ_MAX = int(gq_size.max())
    OHWQ_MAX = int((oh_t1 - oh_t0).max())

    nc = bacc.Bacc("TRN2", target_bir_lowering=False, debug=False,
                   num_devices=N_CORES, num_swdge_queues=n_queues)

    xT_d = nc.dram_tensor("xT", [D_IN, S], f32, kind="ExternalInput").ap()
    dinv_d = nc.dram_tensor("dinv", [P, NW], f32, kind="ExternalInput").ap()
    idx_d = nc.dram_tensor("idx16", [P, E_PAD // 16], i16,
                           kind="ExternalInput").ap()
    oh_d = nc.dram_tensor("oh8", [P, TILES * P], fp8, kind="ExternalInput").ap()
    ohg_d = nc.dram_tensor("ohg8", [P, NW * GMAX], fp8,
                           kind="ExternalInput").ap()
    W_d = [nc.dram_tensor("W1", [D_IN, H], f32, kind="ExternalInput").ap(),
           nc.dram_tensor("W2", [H, H], f32, kind="ExternalInput").ap(),
           nc.dram_tensor("W3", [H, H], f32, kind="ExternalInput").ap()]
    Wf_d = nc.dram_tensor("Wf", [H, N_CLS], f32, kind="ExternalInput").ap()
    b_d = [nc.dram_tensor(f"b{i+1}b", [P, H], f32, kind="ExternalInput").ap()
           for i in range(3)]
    bf_d = nc.dram_tensor("bfb", [P, N_CLS], f32, kind="ExternalInput").ap()
    ident_d = nc.dram_tensor("ident", [P, P], f32, kind="ExternalInput").ap()
    out_d = nc.dram_tensor("out", [GMAX, N_CLS], f32, kind="ExternalOutput").ap()

    zn_dt = f32 if fp32_ag else bf16
    zn_d = [nc.dram_tensor(f"zn{par}", [S, H], zn_dt).ap() for par in range(2)]
    tb_d = [nc.dram_tensor(f"tb{par}", [N_CORES * S, H], bf16,
                           addr_space="Shared").ap() for par in range(2)]
    tf_d = [[nc.dram_tensor(f"tf{par}q{k}", [N_CORES * QROWS, H], f32,
                            addr_space="Shared").ap()
             for k in range(NQ)] for par in range(2)]
    rg = [list(range(N_CORES))]

    with tile.TileContext(nc) as tc:
        with (
            tc.tile_pool(name="persist", bufs=1) as pp,
            tc.tile_pool(name="xq", bufs=1) as xp,
            tc.tile_pool(name="znq", bufs=2) as znp,
            tc.tile_pool(name="sbt", bufs=3) as sbtp,
            tc.tile_pool(name="idx", bufs=4) as idxp,
            tc.tile_pool(name="msgf", bufs=3) as msgfp,
            tc.tile_pool(name="msgb", bufs=6) as msgbp,
            tc.tile_pool(name="oh", bufs=2) as ohp,
            tc.tile_pool(name="wt", bufs=4) as wtp,
            tc.tile_pool(name="xb", bufs=3) as xbp,
            tc.tile_pool(name="xo", bufs=3) as xop,
            tc.tile_pool(name="dps", bufs=2, space="PSUM") as dps,
            tc.tile_pool(name="zps", bufs=2, space="PSUM") as zpsp,
            tc.tile_pool(name="wps", bufs=3, space="PSUM") as wpsp,
            tc.tile_pool(name="pps", bufs=1, space="PSUM") as poolps,
        ):
            nc.gpsimd.load_library(library_config.mlp)
            ident = pp.tile([P, P], f32, tag="ident")
            nc.sync.dma_start(ident[:], ident_d[:])
            W_sb = []
            for i in range(3):
                k = D_IN if i == 0 else H
                t = pp.tile([k, H], f32, tag=f"W{i}")
                nc.sync.dma_start(t[:], W_d[i][:])
                W_sb.append(t)
            Wf_sb = pp.tile([H, N_CLS], f32, tag="Wf")
            nc.sync.dma_start(Wf_sb[:], Wf_d[:])
            b_sb = []
            for i in range(3):
                t = pp.tile([P, H], f32, tag=f"b{i}")
                nc.sync.dma_start(t[:], b_d[i][:])
                b_sb.append(t)
            bf_sb = pp.tile([P, N_CLS], f32, tag="bf")
            nc.sync.dma_start(bf_sb[:], bf_d[:])
            dinv = pp.tile([P, NW], f32, tag="dinv")
            nc.sync.dma_start(dinv[:], dinv_d[:])

            bufA = pp.tile([P, NW * H], f32, tag="bufA")
            bufP = pp.tile([P, NW * H], bf16, tag="bufP")

            def emit_dense_window(layer, w, znq, wl, xTq):
                zps = zpsp.tile([P, H], f32, tag="zps")
                if layer == 0:
                    nc.tensor.matmul(
                        zps[:], lhsT=xTq[:, wl * P:(wl + 1) * P],
                        rhs=W_sb[0][:], start=True, stop=True)
                else:
                    tps = dps.tile([H, P], f32, tag="tps")
                    nc.tensor.transpose(
                        tps[:], bufA[:, w * H:(w + 1) * H], ident[:])
                    sbT = sbtp.tile([H, P], f32, tag="sbT")
                    nc.scalar.activation(sbT[:], tps[:], AOT.Copy)
                    nc.tensor.matmul(
                        zps[:], lhsT=sbT[:], rhs=W_sb[layer][:],
                        start=True, stop=True)
                nc.scalar.activation(
                    znq[:, wl * H:(wl + 1) * H], zps[:], AOT.Copy,
                    scale=dinv[:, w:w + 1])

            def emit_expand(par, k):
                # bf16 -> f32 expansion via SBUF bounce: HWDGE in, DVE cast,
                # HWDGE out (keeps Pool free for gathers/collectives)
                for c in range(N_CORES):
                    s0 = c * S + k * QROWS
                    tin = xbp.tile([P, WPQ * H], bf16, tag="tin")
                    nc.sync.dma_start(
                        tin[:].rearrange("p (t d) -> p t d", d=H),
                        tb_d[par][s0:s0 + QROWS, :].rearrange(
                            "(t p) d -> p t d", p=P))
                    tout = xop.tile([P, WPQ * H], f32, tag="tout")
                    nc.vector.tensor_copy(tout[:], tin[:])
                    nc.sync.dma_start(
                        tf_d[par][k][c * QROWS:(c + 1) * QROWS, :].rearrange(
                            "(t p) d -> p t d", p=P),
                        tout[:].rearrange("p (t d) -> p t d", d=H))

            def emit_gather(par, wq):
                msgB = {}
                for k in range(NQ):
                    gsz = int(gq_size[k, wq])
                    if gsz == 0:
                        continue
                    g0 = int(gq_start[k, wq])
                    idxt = idxp.tile([P, gsz // 16], i16, tag="idxt")
                    nc.sync.dma_start(
                        idxt[:], idx_d[:, g0 // 16:(g0 + gsz) // 16])
                    mf = msgfp.tile([P, (gsz // P) * H], f32, tag="mf")
                    if do_gather:
                        nc.gpsimd.dma_gather(
                            mf[:].rearrange("p (t d) -> p t d", d=H),
                            tf_d[par][k][:],
                            idxt[:], gsz, gsz, H,
                            single_packet=single_packet,
                            queue_num=k % n_queues)
                    else:
                        nc.sync.dma_start(
                            mf[:].rearrange("p (t d) -> p t d", d=H),
                            tf_d[par][k][0:gsz, :].rearrange(
                                "(t p) d -> p t d", p=P))
                    mb_ = msgbp.tile([P, (gsz // P) * H], bf16, tag="mb")
                    nc.scalar.activation(mb_[:], mf[:], AOT.Copy)
                    msgB[k] = mb_
                return msgB

            def emit_scatter_window(layer, wq, w, msgB, oht, t0):
                ntiles = sum(int(tiles_kw[k, w]) for k in range(NQ))
                if not do_scatter:
                    ntiles = 0
                sl = bufA[:, w * H:(w + 1) * H]
                if ntiles:
                    wp = wpsp.tile([P, H], f32, tag="wp")
                    done = 0
                    for k in range(NQ):
                        lb = (int(goff[k, w]) - int(gq_start[k, wq])) // P
                        for i in range(int(tiles_kw[k, w])):
                            T = int(sc_tile[k, w]) + i
                            nc.tensor.matmul(
                                wp[:],
                                lhsT=oht[:, (T - t0) * P:(T - t0 + 1) * P],
                                rhs=msgB[k][:, (lb + i) * H:(lb + i + 1) * H],
                                start=(done == 0), stop=(done == ntiles - 1))
                            done += 1
                    acc = wp[:]
                else:
                    zt = wtp.tile([P, H], f32, tag="ft")
                    nc.vector.memset(zt[:], 0.0)
                    acc = zt[:]
                if layer < 2:
                    nc.vector.scalar_tensor_tensor(
                        out=sl, in0=acc, scalar=dinv[:, w:w + 1],
                        in1=b_sb[layer][:], op0=ALU.mult, op1=ALU.add)
                    nc.scalar.activation(sl, sl, AOT.Tanh)
                else:
                    ft = wtp.tile([P, H], f32, tag="ft")
                    nc.vector.scalar_tensor_tensor(
                        out=ft[:], in0=acc, scalar=dinv[:, w:w + 1],
                        in1=b_sb[layer][:], op0=ALU.mult, op1=ALU.add)
                    nc.scalar.activation(
                        bufP[:, w * H:(w + 1) * H], ft[:], AOT.Tanh)

            def emit_layer(layer):
                par = layer % 2
                # ---- dense + publish, quarter-major ----
                for q in range(NQ):
                    xTq = None
                    if layer == 0:
                        xTq = xp.tile([D_IN, QROWS], f32, tag="xTq")
                        nc.sync.dma_start(
                            xTq[:], xT_d[:, q * QROWS:(q + 1) * QROWS])
                    znq = znp.tile([P, WPQ * H], zn_dt, tag="znq")
                    for wl in range(WPQ):
                        emit_dense_window(layer, q * WPQ + wl, znq, wl, xTq)
                    nc.sync.dma_start(
                        zn_d[par][q * QROWS:(q + 1) * QROWS, :].rearrange(
                            "(t p) d -> p t d", p=P),
                        znq[:].rearrange("p (t d) -> p t d", d=H))
                    if fp32_ag and do_ag:
                        nc.gpsimd.collective_compute(
                            "AllGather", ALU.bypass, replica_groups=rg,
                            ins=[zn_d[par][q * QROWS:(q + 1) * QROWS, :]],
                            outs=[tf_d[par][q][:]])
                if not fp32_ag:
                    if do_ag:
                        nc.gpsimd.collective_compute(
                            "AllGather", ALU.bypass, replica_groups=rg,
                            ins=[zn_d[par][:]], outs=[tb_d[par][:]])
                    if do_expand:
                        for k in range(NQ):
                            emit_expand(par, k)

                # ---- sparse: single pass over all 4 src chunks ----
                for wq in range(NWQ):
                    msgB = emit_gather(par, wq)
                    t0, t1 = int(oh_t0[wq]), int(oh_t1[wq])
                    oht = None
                    if t1 > t0:
                        oht = ohp.tile([P, (t1 - t0) * P], fp8, tag="oht")
                        nc.sync.dma_start(oht[:], oh_d[:, t0 * P:t1 * P])
                    for w in wq_win[wq]:
                        emit_scatter_window(layer, wq, w, msgB, oht, t0)

            for _rep in range(reps):
                for layer in range(3):
                    emit_layer(layer)

                # ---- pooling ----
                poolT = poolps.tile([H, GMAX], f32, tag="poolT")
                PW = 5
                for q in range(NW // PW):
                    ohgt = ohp.tile([P, PW * GMAX], fp8, tag="ohgt")
                    nc.sync.dma_start(
                        ohgt[:], ohg_d[:, q * PW * GMAX:(q + 1) * PW * GMAX])
                    for wl in range(PW):
                        w = q * PW + wl
                        nc.tensor.matmul(
                            poolT[:], lhsT=bufP[:, w * H:(w + 1) * H],
                            rhs=ohgt[:, wl * GMAX:(wl + 1) * GMAX],
                            start=(w == 0), stop=(w == NW - 1))
                poolS = pp.tile([H, GMAX], f32, tag="poolS")
                nc.scalar.activation(poolS[:], poolT[:], AOT.Copy)

                # ---- head ----
                for gt in range(GMAX // P):
                    fps = wpsp.tile([P, N_CLS], f32, tag="wp")
                    nc.tensor.matmul(
                        fps[:], lhsT=poolS[:, gt * P:(gt + 1) * P],
                        rhs=Wf_sb[:], start=True, stop=True)
                    ot = wtp.tile([P, N_CLS], f32, tag="ot")
                    nc.vector.tensor_tensor(out=ot[:], in0=fps[:],
                                            in1=bf_sb[:], op=ALU.add)
                    nc.scalar.activation(ot[:], ot[:], AOT.Tanh)
                    nc.sync.dma_start(out_d[gt * P:(gt + 1) * P, :], ot[:])

    nc.compile()
    return nc


# ----------------------------------------------------------------------------
# Runner
# ----------------------------------------------------------------------------

class Runner:
    def __init__(self, meta, nc):
        self.meta = meta
        self.nc = nc

    def in_maps(self, x, W1, b1, W2, b2, W3, b3, Wf, bf):
        m = self.meta
        S = m["S"]
        C = N_CORES
        x = np.asarray(x, np.float32)
        maps = []
        reps = dict(
            W1=np.asarray(W1, np.float32), W2=np.asarray(W2, np.float32),
            W3=np.asarray(W3, np.float32), Wf=np.asarray(Wf, np.float32),
            b1b=np.broadcast_to(np.asarray(b1, np.float32), (P, H)).copy(),
            b2b=np.broadcast_to(np.asarray(b2, np.float32), (P, H)).copy(),
            b3b=np.broadcast_to(np.asarray(b3, np.float32), (P, H)).copy(),
            bfb=np.broadcast_to(np.asarray(bf, np.float32), (P, N_CLS)).copy(),
            ident=np.eye(P, dtype=np.float32),
        )
        for c in range(C):
            n0, n1 = int(m["node_bnds"][c]), int(m["node_bnds"][c + 1])
            xT = np.zeros((D_IN, S), np.float32)
            xT[:, : n1 - n0] = x[n0:n1].T
            maps.append(dict(
                xT=xT,
                dinv=m["dinv_col"][c],
                idx16=m["idx_tiled"][c],
                oh8=m["oh8"][c],
                ohg8=m["ohg8"][c],
                **reps,
            ))
        return maps

    def run(self, maps):
        from concourse.bass_utils import run_bass_kernel_spmd
        res = run_bass_kernel_spmd(self.nc, maps, list(range(N_CORES)))
        return self.assemble(res.results)

    def make_timed(self, maps):
        import jax
        import concourse.mybir as mybir
        from concourse import bass2jax
        from jax.experimental.shard_map import shard_map
        from jax.sharding import Mesh, NamedSharding, PartitionSpec

        nc = self.nc
        bass2jax.install_neuronx_cc_hook()
        partition_name = (nc.partition_id_tensor.name
                          if nc.partition_id_tensor else None)
        in_names, out_names, out_avals, zero_outs = [], [], [], []
        for alloc in nc.m.functions[0].allocations:
            if not isinstance(alloc, mybir.MemoryLocationSet):
                continue
            name = alloc.memorylocations[0].name
            if alloc.kind == "ExternalInput":
                if name != partition_name:
                    in_names.append(name)
            elif alloc.kind == "ExternalOutput":
                shape = tuple(alloc.tensor_shape)
                dtype = mybir.dt.np(alloc.dtype)
                out_names.append(name)
                out_avals.append(jax.core.ShapedArray(shape, dtype))
                zero_outs.append(np.zeros(shape, dtype))
        n_params = len(in_names)
        all_in = list(in_names) + list(out_names)
        if partition_name is not None:
            all_in.append(partition_name)
        donate = tuple(range(n_params, n_params + len(out_names)))

        def _body(*args):
            operands = list(args)
            if partition_name is not None:
                operands.append(bass2jax.partition_id_tensor())
            return tuple(bass2jax._bass_exec_p.bind(
                *operands, out_avals=tuple(out_avals), in_names=tuple(all_in),
                out_names=tuple(out_names), lowering_input_output_aliases=(),
                sim_require_finite=True, sim_require_nnan=True, nc=nc))

        devices = jax.devices()[:N_CORES]
        mesh = Mesh(np.asarray(devices), ("core",))
        spec = NamedSharding(mesh, PartitionSpec("core"))
        fn = jax.jit(shard_map(_body, mesh=mesh,
                               in_specs=(PartitionSpec("core"),) * (n_params + len(out_names)),
                               out_specs=(PartitionSpec("core"),) * len(out_names)),
                     donate_argnums=donate, keep_unused=True)
        dev_in = [jax.device_put(
            np.concatenate([np.asarray(maps[c][nm]) for c in range(N_CORES)],
                           axis=0), spec) for nm in in_names]
        zshapes = [(N_CORES * z.shape[0], *z.shape[1:]) for z in zero_outs]
        zdtypes = [z.dtype for z in zero_outs]

        def zset():
            return [jax.device_put(np.zeros(s, d), spec)
                    for s, d in zip(zshapes, zdtypes)]

        def call(zs=None):
            zs = zs if zs is not None else zset()
            outs = fn(*dev_in, *zs)
            return [o.block_until_ready() for o in outs]

        return call, zset, out_names

    def assemble(self, results):
        m = self.meta
        outs = []
        for c in range(N_CORES):
            outs.append(results[c]["out"][: int(m["g_cnt"][c])])
        return np.concatenate(outs, axis=0)


_CACHE = {}


def _get_runner(edge_index, batch, n_nodes, n_graphs, reps=1):
    key = (hashlib.sha1(np.ascontiguousarray(edge_index).tobytes()).hexdigest(),
           hashlib.sha1(np.ascontiguousarray(batch).tobytes()).hexdigest(),
           n_nodes, n_graphs, reps)
    r = _CACHE.get(key)
    if r is None:
        meta = _preprocess(edge_index, batch, n_nodes, n_graphs)
        nc = _build_program(meta, reps=reps)
        r = Runner(meta, nc)
        _CACHE[key] = r
    return r


def kernel(x, edge_index, batch, W1, b1, W2, b2, W3, b3, Wf, bf):
    x = np.asarray(x)
    r = _get_runner(np.asarray(edge_index), np.asarray(batch), x.shape[0],
                    N_GRAPHS)
    maps = r.in_maps(x, W1, b1, W2, b2, W3, b3, Wf, bf)
    return r.run(maps)
